# revision 29
# baseline (speedup 1.0000x reference)
"""Trainium2 Bass kernel for nn_Custom_Final_Pooling_2D (segment_reduce).

Computes out = einsum("rn,bn->br", T, x*x) where T is the fixed binary
2x2-pooling selector built by the reference's build_pooling_matrix(32, 16):
  - T has shape [496, 1024]; only rows r0(l)+c are nonzero, where
    r0(l) = 31*l - l*(l+1)//2 + 15, for l, c in [0, 16).
  - Row r0(l)+c sums x[.., i*32+j]^2 over the 2x2 window
    i in {2l, 2l+1}, j in {2c, 2c+1}.

So the kernel is: square (ScalarE, in place), pairwise add along j
(VectorE, stride-2), pairwise add along i (VectorE) into the dense
[rows, 256] pool result — downcast to fp16 on the DVE write — then a
contiguous DMA store of that dense fp16 layout. The host upcasts to
f32, scatters the 16 column blocks to offsets r0(l), and materializes
the 240 always-zero columns while gathering.

Why this shape: the kernel is DMA-bound. The input stream alone
(33.55 MB/core f32) runs at ~345 GB/s — the per-core HBM roofline —
so output bytes cannot hide behind it; they add time linearly. Dense
256-col layout drops 48% of the 496-wide bytes, and fp16 halves the
rest (max elem rel err 4.9e-4, far inside the 2e-2 gate).

Tuning (all measured on HW via the repeat-slope method, interleaved
A/B to beat ±6% device drift):
  - r=8 supertiles (1024 rows) are the sweet spot: per-DMA/instruction
    fixed cost (~1.5-2 us) punishes smaller chunks; bigger chunks blow
    SBUF or lengthen the drain tail.
  - tail_rs=(4,2,2): the last supertile is split so the end-of-pass
    compute+store tail shrinks from ~15 us to ~4 us.
  - y2_bufs=5: lets several output stores ride concurrent DMA channels
    instead of serializing behind one in-flight store.
  - store fusion / multi-queue stores / split stores all measured
    neutral-to-worse; single gpsimd store queue wins.
Per-pass ~112-119 us vs a ~105 us theoretical floor (37.75 MB at
358 GB/s), from a 134 us starting point.

Data-parallel over 8 NeuronCores: batch dim sharded 65536 -> 8 x 8192.
"""

import numpy as np

import concourse.bacc as bacc
import concourse.mybir as mybir
from concourse.tile import TileContext
from concourse.bass_utils import run_bass_kernel_spmd

N_CORES = 8
BATCH = 65536
IMG = 32          # input image side
OUT_SIDE = 16     # pooled side
N_FEAT = IMG * IMG          # 1024
N_OUT = (2 * OUT_SIDE) * (2 * OUT_SIDE - 1) // 2  # 496
ROWS_PER_CORE = BATCH // N_CORES  # 8192

P = 128           # SBUF partitions
R = 8             # batch rows per partition per supertile
SUPER = P * R     # 1024 batch rows per supertile
N_TILES = ROWS_PER_CORE // SUPER  # 8

# Nonzero-row offsets of T: line l's 16 outputs live at columns
# r0(l) .. r0(l)+15 of the 496-wide output.
R0 = [31 * l - l * (l + 1) // 2 + 15 for l in range(OUT_SIDE)]

N_ACT = OUT_SIDE * OUT_SIDE           # 256

_CACHE = {}


def build_program(rows: int = ROWS_PER_CORE, r: int = R, repeat: int = 1,
                  internal_io: bool = False, mode: str = "full",
                  out_dt=None, unroll: int = 1, store_fuse: int = 1,
                  xin_bufs: int = 3, y1_bufs: int = 2, y2_bufs: int = 5,
                  in_engines=("sync",), out_engine: str = "gpsimd",
                  tail_rs=(4, 2, 2), add2_engine: str = "vector",
                  store_split: int = 1, compute_split: int = 1,
                  y1_inplace: bool = False):
    """Build the per-core Bass program: x [rows, 1024] -> out [rows, 256].

    repeat > 1 wraps the whole body in a hardware For_i loop that redoes
    the identical pass `repeat` times — used only for benchmarking (the
    slope over `repeat` isolates on-device time from host overhead).
    `unroll` replicates the body inside the loop (repeat % unroll == 0),
    to diagnose loop-boundary pipeline drain.

    internal_io=True replaces the I/O tensors with internal DRAM buffers
    (plus a dummy [1,1] external output) so benchmark calls skip the
    256 MiB host<->device transfer entirely. The instruction stream is
    identical to the real program.

    mode: "full" (real kernel) | "in_only" | "out_only" (DMA stream
    probes for benchmarking).

    store_fuse: accumulate this many consecutive chunks' pool results in
    one SBUF tile and store them with a single (strided) DMA.
    """
    if out_dt is None:
        out_dt = mybir.dt.float16
    nc = bacc.Bacc("TRN2", target_bir_lowering=False, debug=False,
                   num_devices=N_CORES)
    f32 = mybir.dt.float32
    if internal_io:
        x = nc.dram_tensor("xbuf", [rows, N_FEAT], f32).ap()
        out = nc.dram_tensor("obuf", [rows, N_ACT], out_dt).ap()
        dummy = nc.dram_tensor("out", [1, 1], f32, kind="ExternalOutput").ap()
    else:
        x = nc.dram_tensor("x", [rows, N_FEAT], f32,
                           kind="ExternalInput").ap()
        out = nc.dram_tensor("out", [rows, N_ACT], out_dt,
                             kind="ExternalOutput").ap()

    r_units = rows // P
    tail_rs = [int(v) for v in tail_rs]
    n_full = (r_units - sum(tail_rs)) // r
    assert n_full * r + sum(tail_rs) == r_units, (r, tail_rs)
    chunk_rs = [r] * n_full + tail_rs
    n_chunks = len(chunk_rs)

    # Store groups: fuse `store_fuse` consecutive full-size chunks per
    # store DMA; tail chunks store individually. Each chunk gets
    # (group_size, idx_in_group).
    groups = []
    i = 0
    while i < n_chunks:
        if chunk_rs[i] == r:
            g = 1
            while (g < store_fuse and i + g < n_chunks
                   and chunk_rs[i + g] == r):
                g += 1
        else:
            g = 1
        groups.append((i, g))
        i += g
    chunk_group = {}
    for gstart, gsize in groups:
        for k in range(gsize):
            chunk_group[gstart + k] = (gsize, k)

    # Per chunk: partition p holds rr consecutive batch rows.
    def x_view(row0, rr):
        return x[row0:row0 + P * rr].rearrange("(p r) m -> p (r m)",
                                               p=P, r=rr)

    def o_view(row0, rr):
        return out[row0:row0 + P * rr].rearrange("(p r) m -> p (r m)",
                                                 p=P, r=rr)

    def o_view_fused(row0, rr, nf):
        # nf consecutive chunks in one AP: partition p covers, for each
        # chunk c, rows row0 + c*P*rr + p*rr ... + rr  (2D per-partition
        # access: nf blocks of rr*N_ACT, DRAM-strided by P*rr*N_ACT)
        return out[row0:row0 + nf * P * rr].rearrange(
            "(c p r) m -> p c (r m)", c=nf, p=P, r=rr)

    in_eng = [getattr(nc, e) for e in in_engines]
    if isinstance(out_engine, str):
        out_engine = (out_engine,)
    out_engs = [getattr(nc, e) for e in out_engine]

    with TileContext(nc) as tc:
        with tc.tile_pool(name="xin", bufs=xin_bufs) as xin_pool, \
             tc.tile_pool(name="y1", bufs=y1_bufs) as y1_pool, \
             tc.tile_pool(name="y2", bufs=y2_bufs) as y2_pool:
            if internal_io:
                # zero-fill the internal input region once so the bench
                # never squares NaN/Inf garbage, and feed the dummy output
                zt = xin_pool.tile([P, r * N_FEAT], f32, tag="xt")
                nc.gpsimd.memset(zt[:], 0.0)
                row0 = 0
                for rr in chunk_rs:
                    nc.sync.dma_start(out=x_view(row0, rr),
                                      in_=zt[:, :rr * N_FEAT])
                    row0 += P * rr
                nc.sync.dma_start(out=dummy, in_=zt[:1, :1])
                if mode == "out_only":
                    zo = y2_pool.tile([P, store_fuse * r * N_ACT], out_dt,
                                      tag="y2")
                    nc.gpsimd.memset(zo[:], 0.0)

            def body():
                row0 = 0
                for t, rr in enumerate(chunk_rs):
                    if mode == "out_only":
                        gsize, fi = chunk_group[t]
                        if fi == 0:
                            out_engs[t % len(out_engs)].dma_start(
                                out=o_view_fused(row0, rr, gsize),
                                in_=zo[:, :gsize * rr * N_ACT]
                                .rearrange("p (c rm) -> p c rm", c=gsize))
                        row0 += P * rr
                        continue
                    xt = xin_pool.tile([P, rr * N_FEAT], f32, tag="xt")
                    in_eng[t % len(in_eng)].dma_start(out=xt[:],
                                                      in_=x_view(row0, rr))
                    if mode == "in_only":
                        row0 += P * rr
                        continue

                    if compute_split > 1 and rr == r:
                        # big load, fine-grained compute: process the
                        # tile in sub-pieces so the xin buffer frees
                        # quickly and 2 deep buffers suffice
                        assert rr % compute_split == 0
                        rs = rr // compute_split
                        for s in range(compute_split):
                            xs = xt[:, s * rs * N_FEAT:(s + 1) * rs * N_FEAT]
                            nc.scalar.activation(
                                xs, xs, mybir.ActivationFunctionType.Square)
                            if y1_inplace:
                                y1ap = xs[:, :rs * N_FEAT // 2]
                            else:
                                y1t = y1_pool.tile(
                                    [P, rs * N_FEAT // 2], f32, tag="y1")
                                y1ap = y1t[:]
                            nc.vector.tensor_add(y1ap, xs[:, 0::2],
                                                 xs[:, 1::2])
                            y1v = y1ap.rearrange(
                                "p (row l two c) -> p row l two c",
                                row=rs, l=OUT_SIDE, two=2, c=OUT_SIDE)
                            y2 = y2_pool.tile([P, rs * N_ACT], out_dt,
                                              tag="y2")
                            y2v = y2[:].rearrange(
                                "p (row l c) -> p row l c",
                                row=rs, l=OUT_SIDE, c=OUT_SIDE)
                            getattr(nc, add2_engine).tensor_add(
                                y2v, y1v[:, :, :, 0, :], y1v[:, :, :, 1, :])
                            # partition p holds rows row0+rr*p .. +rr;
                            # sub-piece s covers local rows s*rs..(s+1)*rs
                            dst = out[row0:row0 + P * rr].rearrange(
                                "(p r) m -> p r m", p=P,
                                r=rr)[:, s * rs:(s + 1) * rs, :]
                            out_engs[t % len(out_engs)].dma_start(
                                out=dst,
                                in_=y2[:].rearrange("p (r m) -> p r m",
                                                    r=rs))
                        row0 += P * rr
                        continue

                    # square in place (elementwise, same AP — safe)
                    nc.scalar.activation(xt[:], xt[:],
                                         mybir.ActivationFunctionType.Square)

                    # pool over j: y1[p, 512rr], index = 512*row + 16*i + c
                    if y1_inplace:
                        # write into xt's own first half: the DVE write
                        # pointer (q) always trails its read pointer (2q),
                        # so the overlap is hazard-free and saves the y1
                        # pool's 32 KB of SBUF
                        y1ap = xt[:, :rr * N_FEAT // 2]
                    else:
                        y1t = y1_pool.tile([P, rr * N_FEAT // 2], f32,
                                           tag="y1")
                        y1ap = y1t[:]
                    nc.vector.tensor_add(y1ap, xt[:, 0::2], xt[:, 1::2])

                    # pool over i: one dense add into (a slice of) y2,
                    # downcast to out_dt on the DVE write
                    # (y1 viewed [p, row, l, two, c]; y2 = even + odd i)
                    y1v = y1ap.rearrange("p (row l two c) -> p row l two c",
                                         row=rr, l=OUT_SIDE, two=2,
                                         c=OUT_SIDE)
                    gsize, fi = chunk_group[t]
                    if fi == 0:
                        y2 = y2_pool.tile([P, gsize * rr * N_ACT],
                                          out_dt, tag="y2")
                    y2s = y2[:, fi * rr * N_ACT:(fi + 1) * rr * N_ACT]
                    y2v = y2s.rearrange("p (row l c) -> p row l c",
                                        row=rr, l=OUT_SIDE, c=OUT_SIDE)
                    getattr(nc, add2_engine).tensor_add(
                        y2v, y1v[:, :, :, 0, :], y1v[:, :, :, 1, :])

                    # contiguous dense store, issued from an otherwise-
                    # idle engine's queue so its wait-for-DVE never
                    # stalls another sequencer
                    if fi == gsize - 1:
                        srow0 = row0 - (gsize - 1) * P * rr
                        dst = o_view_fused(srow0, rr, gsize)
                        src = y2[:].rearrange("p (c rm) -> p c rm", c=gsize)
                        if store_split == 1:
                            out_engs[t % len(out_engs)].dma_start(
                                out=dst, in_=src)
                        else:
                            # split each store across queues for more
                            # DMA-channel parallelism per store
                            h = rr * N_ACT // 2
                            for si in range(2):
                                out_engs[(t + si) % len(out_engs)].dma_start(
                                    out=dst[:, :, si * h:(si + 1) * h],
                                    in_=src[:, :, si * h:(si + 1) * h])
                    row0 += P * rr

            if repeat == 1:
                body()
            else:
                assert repeat % unroll == 0, (repeat, unroll)
                with tc.For_i(0, repeat // unroll, 1):
                    for _ in range(unroll):
                        body()

    nc.compile()
    return nc


def kernel(**inputs) -> np.ndarray:
    x = np.ascontiguousarray(inputs["input_state"], dtype=np.float32)
    assert x.shape == (BATCH, N_FEAT), x.shape

    if "nc" not in _CACHE:
        _CACHE["nc"] = build_program()
    nc = _CACHE["nc"]

    shards = [x[i * ROWS_PER_CORE:(i + 1) * ROWS_PER_CORE]
              for i in range(N_CORES)]
    in_maps = [{"x": s} for s in shards]
    res = run_bass_kernel_spmd(nc, in_maps, list(range(N_CORES)))

    # gather + unshard: upcast fp16 -> f32, scatter the dense 16-col
    # blocks to R0[l], and materialize the always-zero columns host-side
    compact = np.concatenate([res.results[i]["out"] for i in range(N_CORES)],
                             axis=0)
    full = np.zeros((BATCH, N_OUT), dtype=np.float32)
    for l in range(OUT_SIDE):
        full[:, R0[l]:R0[l] + OUT_SIDE] = \
            compact[:, l * OUT_SIDE:(l + 1) * OUT_SIDE]
    return full


# revision 30
# speedup vs baseline: 1.0140x; 1.0140x over previous
"""Trainium2 Bass kernel for nn_Custom_Final_Pooling_2D (segment_reduce).

Computes out = einsum("rn,bn->br", T, x*x) where T is the fixed binary
2x2-pooling selector built by the reference's build_pooling_matrix(32, 16):
  - T has shape [496, 1024]; only rows r0(l)+c are nonzero, where
    r0(l) = 31*l - l*(l+1)//2 + 15, for l, c in [0, 16).
  - Row r0(l)+c sums x[.., i*32+j]^2 over the 2x2 window
    i in {2l, 2l+1}, j in {2c, 2c+1}.

So the kernel is: square (ScalarE, in place), pairwise add along j
(VectorE, stride-2), pairwise add along i (VectorE) into the dense
[rows, 256] pool result — downcast to fp16 on the DVE write — then a
contiguous DMA store of that dense fp16 layout. The host upcasts to
f32, scatters the 16 column blocks to offsets r0(l), and materializes
the 240 always-zero columns while gathering.

Why this shape: the kernel is DMA-bound. The input stream alone
(33.55 MB/core f32) runs at ~345 GB/s — the per-core HBM roofline —
so output bytes cannot hide behind it; they add time linearly. Dense
256-col layout drops 48% of the 496-wide bytes, and fp16 halves the
rest (max elem rel err 4.9e-4, far inside the 2e-2 gate).

Tuning (all measured on HW via the repeat-slope method, interleaved
A/B to beat ±6% device drift):
  - r=8 supertiles (1024 rows) are the sweet spot: per-DMA/instruction
    fixed cost (~1.5-2 us) punishes smaller chunks; bigger chunks blow
    SBUF or lengthen the drain tail.
  - tail_rs=(4,2,2): the last supertile is split so the end-of-pass
    compute+store tail shrinks from ~15 us to ~4 us.
  - y2_bufs=5: lets several output stores ride concurrent DMA channels
    instead of serializing behind one in-flight store.
  - store fusion / multi-queue stores / split stores all measured
    neutral-to-worse; single gpsimd store queue wins.
Per-pass ~112-119 us vs a ~105 us theoretical floor (37.75 MB at
358 GB/s), from a 134 us starting point.

Data-parallel over 8 NeuronCores: batch dim sharded 65536 -> 8 x 8192.
"""

import numpy as np

import concourse.bacc as bacc
import concourse.mybir as mybir
from concourse.tile import TileContext
from concourse.bass_utils import run_bass_kernel_spmd

N_CORES = 8
BATCH = 65536
IMG = 32          # input image side
OUT_SIDE = 16     # pooled side
N_FEAT = IMG * IMG          # 1024
N_OUT = (2 * OUT_SIDE) * (2 * OUT_SIDE - 1) // 2  # 496
ROWS_PER_CORE = BATCH // N_CORES  # 8192

P = 128           # SBUF partitions
R = 8             # batch rows per partition per supertile
SUPER = P * R     # 1024 batch rows per supertile
N_TILES = ROWS_PER_CORE // SUPER  # 8

# Nonzero-row offsets of T: line l's 16 outputs live at columns
# r0(l) .. r0(l)+15 of the 496-wide output.
R0 = [31 * l - l * (l + 1) // 2 + 15 for l in range(OUT_SIDE)]

N_ACT = OUT_SIDE * OUT_SIDE           # 256

_CACHE = {}


def build_program(rows: int = ROWS_PER_CORE, r: int = R, repeat: int = 1,
                  internal_io: bool = False, mode: str = "full",
                  out_dt=None, unroll: int = 1, store_fuse: int = 1,
                  xin_bufs: int = 3, y1_bufs: int = 2, y2_bufs: int = 5,
                  in_engines=("sync",), out_engine: str = "gpsimd",
                  tail_rs=(3, 3, 2), add2_engine: str = "vector",
                  store_split: int = 1, compute_split: int = 1,
                  y1_inplace: bool = False):
    """Build the per-core Bass program: x [rows, 1024] -> out [rows, 256].

    repeat > 1 wraps the whole body in a hardware For_i loop that redoes
    the identical pass `repeat` times — used only for benchmarking (the
    slope over `repeat` isolates on-device time from host overhead).
    `unroll` replicates the body inside the loop (repeat % unroll == 0),
    to diagnose loop-boundary pipeline drain.

    internal_io=True replaces the I/O tensors with internal DRAM buffers
    (plus a dummy [1,1] external output) so benchmark calls skip the
    256 MiB host<->device transfer entirely. The instruction stream is
    identical to the real program.

    mode: "full" (real kernel) | "in_only" | "out_only" (DMA stream
    probes for benchmarking).

    store_fuse: accumulate this many consecutive chunks' pool results in
    one SBUF tile and store them with a single (strided) DMA.
    """
    if out_dt is None:
        out_dt = mybir.dt.float16
    nc = bacc.Bacc("TRN2", target_bir_lowering=False, debug=False,
                   num_devices=N_CORES)
    f32 = mybir.dt.float32
    if internal_io:
        x = nc.dram_tensor("xbuf", [rows, N_FEAT], f32).ap()
        out = nc.dram_tensor("obuf", [rows, N_ACT], out_dt).ap()
        dummy = nc.dram_tensor("out", [1, 1], f32, kind="ExternalOutput").ap()
    else:
        x = nc.dram_tensor("x", [rows, N_FEAT], f32,
                           kind="ExternalInput").ap()
        out = nc.dram_tensor("out", [rows, N_ACT], out_dt,
                             kind="ExternalOutput").ap()

    r_units = rows // P
    tail_rs = [int(v) for v in tail_rs]
    n_full = (r_units - sum(tail_rs)) // r
    assert n_full * r + sum(tail_rs) == r_units, (r, tail_rs)
    chunk_rs = [r] * n_full + tail_rs
    n_chunks = len(chunk_rs)

    # Store groups: fuse `store_fuse` consecutive full-size chunks per
    # store DMA; tail chunks store individually. Each chunk gets
    # (group_size, idx_in_group).
    groups = []
    i = 0
    while i < n_chunks:
        if chunk_rs[i] == r:
            g = 1
            while (g < store_fuse and i + g < n_chunks
                   and chunk_rs[i + g] == r):
                g += 1
        else:
            g = 1
        groups.append((i, g))
        i += g
    chunk_group = {}
    for gstart, gsize in groups:
        for k in range(gsize):
            chunk_group[gstart + k] = (gsize, k)

    # Per chunk: partition p holds rr consecutive batch rows.
    def x_view(row0, rr):
        return x[row0:row0 + P * rr].rearrange("(p r) m -> p (r m)",
                                               p=P, r=rr)

    def o_view(row0, rr):
        return out[row0:row0 + P * rr].rearrange("(p r) m -> p (r m)",
                                                 p=P, r=rr)

    def o_view_fused(row0, rr, nf):
        # nf consecutive chunks in one AP: partition p covers, for each
        # chunk c, rows row0 + c*P*rr + p*rr ... + rr  (2D per-partition
        # access: nf blocks of rr*N_ACT, DRAM-strided by P*rr*N_ACT)
        return out[row0:row0 + nf * P * rr].rearrange(
            "(c p r) m -> p c (r m)", c=nf, p=P, r=rr)

    in_eng = [getattr(nc, e) for e in in_engines]
    if isinstance(out_engine, str):
        out_engine = (out_engine,)
    out_engs = [getattr(nc, e) for e in out_engine]

    with TileContext(nc) as tc:
        with tc.tile_pool(name="xin", bufs=xin_bufs) as xin_pool, \
             tc.tile_pool(name="y1", bufs=y1_bufs) as y1_pool, \
             tc.tile_pool(name="y2", bufs=y2_bufs) as y2_pool:
            if internal_io:
                # zero-fill the internal input region once so the bench
                # never squares NaN/Inf garbage, and feed the dummy output
                zt = xin_pool.tile([P, r * N_FEAT], f32, tag="xt")
                nc.gpsimd.memset(zt[:], 0.0)
                row0 = 0
                for rr in chunk_rs:
                    nc.sync.dma_start(out=x_view(row0, rr),
                                      in_=zt[:, :rr * N_FEAT])
                    row0 += P * rr
                nc.sync.dma_start(out=dummy, in_=zt[:1, :1])
                if mode == "out_only":
                    zo = y2_pool.tile([P, store_fuse * r * N_ACT], out_dt,
                                      tag="y2")
                    nc.gpsimd.memset(zo[:], 0.0)

            def body():
                row0 = 0
                for t, rr in enumerate(chunk_rs):
                    if mode == "out_only":
                        gsize, fi = chunk_group[t]
                        if fi == 0:
                            out_engs[t % len(out_engs)].dma_start(
                                out=o_view_fused(row0, rr, gsize),
                                in_=zo[:, :gsize * rr * N_ACT]
                                .rearrange("p (c rm) -> p c rm", c=gsize))
                        row0 += P * rr
                        continue
                    xt = xin_pool.tile([P, rr * N_FEAT], f32, tag="xt")
                    in_eng[t % len(in_eng)].dma_start(out=xt[:],
                                                      in_=x_view(row0, rr))
                    if mode == "in_only":
                        row0 += P * rr
                        continue

                    if compute_split > 1 and rr == r:
                        # big load, fine-grained compute: process the
                        # tile in sub-pieces so the xin buffer frees
                        # quickly and 2 deep buffers suffice
                        assert rr % compute_split == 0
                        rs = rr // compute_split
                        for s in range(compute_split):
                            xs = xt[:, s * rs * N_FEAT:(s + 1) * rs * N_FEAT]
                            nc.scalar.activation(
                                xs, xs, mybir.ActivationFunctionType.Square)
                            if y1_inplace:
                                y1ap = xs[:, :rs * N_FEAT // 2]
                            else:
                                y1t = y1_pool.tile(
                                    [P, rs * N_FEAT // 2], f32, tag="y1")
                                y1ap = y1t[:]
                            nc.vector.tensor_add(y1ap, xs[:, 0::2],
                                                 xs[:, 1::2])
                            y1v = y1ap.rearrange(
                                "p (row l two c) -> p row l two c",
                                row=rs, l=OUT_SIDE, two=2, c=OUT_SIDE)
                            y2 = y2_pool.tile([P, rs * N_ACT], out_dt,
                                              tag="y2")
                            y2v = y2[:].rearrange(
                                "p (row l c) -> p row l c",
                                row=rs, l=OUT_SIDE, c=OUT_SIDE)
                            getattr(nc, add2_engine).tensor_add(
                                y2v, y1v[:, :, :, 0, :], y1v[:, :, :, 1, :])
                            # partition p holds rows row0+rr*p .. +rr;
                            # sub-piece s covers local rows s*rs..(s+1)*rs
                            dst = out[row0:row0 + P * rr].rearrange(
                                "(p r) m -> p r m", p=P,
                                r=rr)[:, s * rs:(s + 1) * rs, :]
                            out_engs[t % len(out_engs)].dma_start(
                                out=dst,
                                in_=y2[:].rearrange("p (r m) -> p r m",
                                                    r=rs))
                        row0 += P * rr
                        continue

                    # square in place (elementwise, same AP — safe)
                    nc.scalar.activation(xt[:], xt[:],
                                         mybir.ActivationFunctionType.Square)

                    # pool over j: y1[p, 512rr], index = 512*row + 16*i + c
                    if y1_inplace:
                        # write into xt's own first half: the DVE write
                        # pointer (q) always trails its read pointer (2q),
                        # so the overlap is hazard-free and saves the y1
                        # pool's 32 KB of SBUF
                        y1ap = xt[:, :rr * N_FEAT // 2]
                    else:
                        y1t = y1_pool.tile([P, rr * N_FEAT // 2], f32,
                                           tag="y1")
                        y1ap = y1t[:]
                    nc.vector.tensor_add(y1ap, xt[:, 0::2], xt[:, 1::2])

                    # pool over i: one dense add into (a slice of) y2,
                    # downcast to out_dt on the DVE write
                    # (y1 viewed [p, row, l, two, c]; y2 = even + odd i)
                    y1v = y1ap.rearrange("p (row l two c) -> p row l two c",
                                         row=rr, l=OUT_SIDE, two=2,
                                         c=OUT_SIDE)
                    gsize, fi = chunk_group[t]
                    if fi == 0:
                        y2 = y2_pool.tile([P, gsize * rr * N_ACT],
                                          out_dt, tag="y2")
                    y2s = y2[:, fi * rr * N_ACT:(fi + 1) * rr * N_ACT]
                    y2v = y2s.rearrange("p (row l c) -> p row l c",
                                        row=rr, l=OUT_SIDE, c=OUT_SIDE)
                    getattr(nc, add2_engine).tensor_add(
                        y2v, y1v[:, :, :, 0, :], y1v[:, :, :, 1, :])

                    # contiguous dense store, issued from an otherwise-
                    # idle engine's queue so its wait-for-DVE never
                    # stalls another sequencer
                    if fi == gsize - 1:
                        srow0 = row0 - (gsize - 1) * P * rr
                        dst = o_view_fused(srow0, rr, gsize)
                        src = y2[:].rearrange("p (c rm) -> p c rm", c=gsize)
                        if store_split == 1:
                            out_engs[t % len(out_engs)].dma_start(
                                out=dst, in_=src)
                        else:
                            # split each store across queues for more
                            # DMA-channel parallelism per store
                            h = rr * N_ACT // 2
                            for si in range(2):
                                out_engs[(t + si) % len(out_engs)].dma_start(
                                    out=dst[:, :, si * h:(si + 1) * h],
                                    in_=src[:, :, si * h:(si + 1) * h])
                    row0 += P * rr

            if repeat == 1:
                body()
            else:
                assert repeat % unroll == 0, (repeat, unroll)
                with tc.For_i(0, repeat // unroll, 1):
                    for _ in range(unroll):
                        body()

    nc.compile()
    return nc


def kernel(**inputs) -> np.ndarray:
    x = np.ascontiguousarray(inputs["input_state"], dtype=np.float32)
    assert x.shape == (BATCH, N_FEAT), x.shape

    if "nc" not in _CACHE:
        _CACHE["nc"] = build_program()
    nc = _CACHE["nc"]

    shards = [x[i * ROWS_PER_CORE:(i + 1) * ROWS_PER_CORE]
              for i in range(N_CORES)]
    in_maps = [{"x": s} for s in shards]
    res = run_bass_kernel_spmd(nc, in_maps, list(range(N_CORES)))

    # gather + unshard: upcast fp16 -> f32, scatter the dense 16-col
    # blocks to R0[l], and materialize the always-zero columns host-side
    compact = np.concatenate([res.results[i]["out"] for i in range(N_CORES)],
                             axis=0)
    full = np.zeros((BATCH, N_OUT), dtype=np.float32)
    for l in range(OUT_SIDE):
        full[:, R0[l]:R0[l] + OUT_SIDE] = \
            compact[:, l * OUT_SIDE:(l + 1) * OUT_SIDE]
    return full
